# Initial kernel scaffold
#
"""Trainium2 Bass kernel for multi-head attention (B=8, N=1024, C=768, H=12, D=64).

Sharding: pure data parallelism — one batch element per NeuronCore (8 cores).
Each core computes qkv projection, softmax attention and output projection for
its [1024, 768] slice with full (replicated) weights. No collectives.

Dataflow (all "transposed" so no on-device transposes are needed):
  - host supplies xT = x[b].T (bf16) and w_qk pre-tiled [H, 128, KT, 128]
  - qk-pass:  qkT = w_qk.T @ x.T -> [1536, 1024]; head-pair t's tiles are
              computed during attention of pair t-1 (PE gap filling)
  - v-pass:   v = x @ w_v natural [1024, 768], computed inside pair 0/2 loops
  - ST pass:  ST[k,q] = (K Q^T) per head
  - exp:      PuT = exp(SCALE * ST) on ScalarE (no max subtraction: |S|<~7)
  - O pass:   O^T[d,q]: lhsT=[v | ones] so PSUM row 64 accumulates the softmax
              denominator l; two q-tile sweeps to keep PSUM pressure low
  - normalize: per head pair and q-tile, O^T *= (1/l); reciprocal read straight
              from the PSUM l-rows, broadcast via one DRAM bounce
  - proj:     yT = w_proj.T @ O^T + b; k-tiles 0..4 pre-accumulated during
              pair 5's attention; the k=5 matmuls for q-tile 0 run during
              sweep B so only q-tile 1 work remains after the last O matmul
  - host transposes yT back.

Schedule notes: input DMAs are issued in first-use order against
fine-grained tiles (dep tracking is tile-granular) so the first qk matmul
starts ~2 us after DMA startup; each pair's j=0 ST+exp is hoisted into the
previous pair's sweep B to kill the boundary exp bubble; proj runs as 12
fused k=0..5 units after pair 5 (qt0 first) so only the k=5 matmuls wait on
the final drains; drain chains ride the idle GpSimd DMA ring and output DMAs
the Scalar ring to avoid head-of-line blocking on Sync.
"""

import sys

sys.path.insert(0, "/opt/trn_rl_repo")

import numpy as np

B, N, C = 8, 1024, 768
H, D = 12, 64
SCALE = D ** -0.5  # 0.125
NCORES = 8
KT = C // 128      # 6 k-tiles over the C contraction
QT = N // 512      # 2 q-tiles of 512
NKT = N // 128     # 8 k-tiles over sequence for attention

_CACHED = None


def _build():
    from contextlib import ExitStack

    from concourse import bacc
    import concourse.bass as bass
    import concourse.mybir as mybir
    from concourse.tile import TileContext
    from bass_rust import add_dep_helper

    f32 = mybir.dt.float32
    bf16 = mybir.dt.bfloat16
    Exp = mybir.ActivationFunctionType.Exp
    Alu = mybir.AluOpType

    nc = bacc.Bacc("TRN2", target_bir_lowering=False, debug=False)

    xT = nc.dram_tensor("xT", [C, N], bf16, kind="ExternalInput").ap()
    wqk = nc.dram_tensor("wqk", [H, 128, KT, 128], bf16, kind="ExternalInput").ap()
    wv = nc.dram_tensor("wv", [C, C], bf16, kind="ExternalInput").ap()
    wp = nc.dram_tensor("wp", [C, C], bf16, kind="ExternalInput").ap()
    bT = nc.dram_tensor("bT", [128, KT], f32, kind="ExternalInput").ap()
    yT = nc.dram_tensor("yT", [C, N], f32, kind="ExternalOutput").ap()
    la_dram = nc.dram_tensor("la_scratch", [H, N], f32).ap()
    ra_dram = nc.dram_tensor("ra_scratch", [H, N], f32).ap()

    with TileContext(nc) as tc, ExitStack() as ctx:
        singles = ctx.enter_context(tc.tile_pool(name="singles", bufs=1))
        put_pool = ctx.enter_context(tc.tile_pool(name="put", bufs=24))
        y_pool = ctx.enter_context(tc.tile_pool(name="y", bufs=3))
        rb_pool = ctx.enter_context(tc.tile_pool(name="rb", bufs=4))
        lst_pool = ctx.enter_context(tc.tile_pool(name="lst", bufs=4))
        la_pool = ctx.enter_context(tc.tile_pool(name="la", bufs=4))
        # PSUM: st (2 tiles x 2 banks) + o (2 x 1) + mm (2 x 1) = 8 banks
        mm_ps = ctx.enter_context(tc.tile_pool(name="mm_ps", bufs=2, space="PSUM"))
        o_ps = ctx.enter_context(tc.tile_pool(name="o_ps", bufs=2, space="PSUM"))
        st_ps = ctx.enter_context(tc.tile_pool(name="st_ps", bufs=2, space="PSUM"))

        # ---- persistent SBUF ----
        # split per (k, q-half) so the first qk matmuls only wait on the
        # slices they read (dependency tracking is tile-granular)
        xT_k = [
            [singles.tile([128, 512], bf16, name=f"xT_k{k}_q{qt}") for qt in range(QT)]
            for k in range(KT)
        ]  # 12 KB/part total
        wqk_sb = [
            singles.tile([128, KT, 128], bf16, name=f"wqk_sb_{m}") for m in range(H)
        ]  # 18 KB/part
        wv_sb = singles.tile([128, KT, C], bf16)          # 9 KB/part
        wp_sb = singles.tile([128, KT, C], bf16)          # 9 KB/part
        bT_sb = singles.tile([128, KT], f32)
        qkT_sb = singles.tile([128, 2 * H, N], bf16)      # 24 KB/part
        v_sb = singles.tile([128, NKT, H, 65], bf16)      # 12.2 KB/part
        # per-(pair, q-half) tiles so proj reads never falsely wait on a
        # later drain (dependency tracking is tile-granular)
        ouT_k = [
            [singles.tile([128, 512], bf16, name=f"ouT_k{t}_q{qt}") for qt in range(QT)]
            for t in range(KT)
        ]  # 12 KB/part total

        # ---- input DMAs in first-use order with fine-grained dep tiles ----
        # first qk unit (t=0,u=0: m=0, qt=0) needs wqk m=0 + xT[:, k, 0:512]
        nc.sync.dma_start(out=xT_k[0][0], in_=xT[0:128, 0:512])
        nc.sync.dma_start(out=wqk_sb[0], in_=wqk[0])
        for k in range(1, KT):
            nc.sync.dma_start(out=xT_k[k][0], in_=xT[k * 128:(k + 1) * 128, 0:512])
        nc.sync.dma_start(out=wqk_sb[6], in_=wqk[6])
        for k in range(KT):
            nc.sync.dma_start(out=xT_k[k][1], in_=xT[k * 128:(k + 1) * 128, 512:1024])
        for k in range(KT):  # wv chunk c0=0 (heads 0-7), used by pair-0 extras
            nc.sync.dma_start(out=wv_sb[:, k, 0:512], in_=wv[k * 128:(k + 1) * 128, 0:512])
        nc.sync.dma_start(out=wqk_sb[1], in_=wqk[1])
        nc.sync.dma_start(out=wqk_sb[7], in_=wqk[7])
        for k in range(KT):  # wv chunk c0=512 (heads 8-11), used by pair-2 extras
            nc.sync.dma_start(out=wv_sb[:, k, 512:768], in_=wv[k * 128:(k + 1) * 128, 512:768])
        for t in range(2, KT):
            nc.sync.dma_start(out=wqk_sb[t], in_=wqk[t])
            nc.sync.dma_start(out=wqk_sb[(H // 2) + t], in_=wqk[(H // 2) + t])
        for k in range(KT):
            nc.sync.dma_start(out=wp_sb[:, k, :], in_=wp[k * 128:(k + 1) * 128, :])
        nc.sync.dma_start(out=bT_sb, in_=bT[:, :])

        # ones column for the softmax-denominator trick
        nc.vector.memset(v_sb[:, :, :, 64:65], 1.0)

        def qk_unit(t, u):
            """One (m, qt) unit of the qk-pass for head pair t (u in 0..3)."""
            m = t if u < 2 else (H // 2) + t
            qt = u % 2
            ps = mm_ps.tile([128, 512], f32, tag="mm", name=f"qk_{m}_{qt}")
            for k in range(KT):
                nc.tensor.matmul(
                    ps,
                    wqk_sb[m][:, k, :],
                    xT_k[k][qt],
                    start=(k == 0),
                    stop=(k == KT - 1),
                )
            nc.vector.tensor_copy(qkT_sb[:, m, qt * 512:(qt + 1) * 512], ps)

        def v_chunk(j, c0, csz):
            """v[n-tile j, c0:c0+csz] = x @ w_v chunk (natural, n on partitions)."""
            ps = mm_ps.tile([128, 512], f32, tag="mm", name=f"v_{j}_{c0}")
            for k in range(KT):
                nc.tensor.matmul(
                    ps[:, 0:csz],
                    xT_k[k][j // 4][:, (j % 4) * 128:(j % 4 + 1) * 128],
                    wv_sb[:, k, c0:c0 + csz],
                    start=(k == 0),
                    stop=(k == KT - 1),
                )
            nh = csz // 64
            nc.vector.tensor_copy(
                v_sb[:, j, c0 // 64:c0 // 64 + nh, 0:64],
                ps[:, 0:csz].rearrange("p (h c) -> p h c", c=64),
            )

        def drain_o(t, qt, tiles):
            """Copy O^T rows to ouT, extract l row, run the (1/l) chain for
            this q-tile half, and normalize the half in place.

            The reciprocal runs on DVE via a DRAM partition-reshape bounce
            (single-lane DVE reciprocal is ~7 ns/elem, so [1,512] is reshaped
            to [8,128] first); the chains ride the idle GpSimd DMA ring so
            they never queue behind input/output traffic."""
            he, ho = 2 * t, 2 * t + 1
            q0 = qt * 512
            la_writes = []
            for h, po in ((he, 0), (ho, 64)):
                nc.vector.tensor_copy(
                    ouT_k[t][qt][po:po + 64, :], tiles[h][0:64, :]
                )
                lst = lst_pool.tile([65, 512], f32, tag="lst", name=f"l_h{h}_q{qt}")
                nc.vector.tensor_copy(lst[64:65, :], tiles[h][64:65, :])
                la_writes.append(nc.gpsimd.dma_start(
                    out=la_dram[h:h + 1, q0:q0 + 512], in_=lst[64:65, :]
                ))
            lv = la_dram[he:he + 2, q0:q0 + 512].rearrange("h (r c) -> h r c", c=128)
            rv = ra_dram[he:he + 2, q0:q0 + 512].rearrange("h (r c) -> h r c", c=128)
            la_t = la_pool.tile([8, 128], f32, tag="la", name=f"la_{t}_{qt}")
            ra_t = la_pool.tile([8, 128], f32, tag="ra", name=f"ra_{t}_{qt}")
            la_rd = nc.gpsimd.dma_start(out=la_t, in_=lv)
            for w in la_writes:
                add_dep_helper(la_rd.ins, w.ins, reason="la dram write->read")
            nc.vector.reciprocal(ra_t, la_t)
            ra_wr = nc.gpsimd.dma_start(out=rv, in_=ra_t)
            rb = rb_pool.tile([128, 512], f32, tag="rb", name=f"rb_{t}_{qt}")
            b1 = nc.gpsimd.dma_start(
                out=rb[0:64, :], in_=ra_dram[he:he + 1, q0:q0 + 512].to_broadcast([64, 512])
            )
            b2 = nc.gpsimd.dma_start(
                out=rb[64:128, :], in_=ra_dram[ho:ho + 1, q0:q0 + 512].to_broadcast([64, 512])
            )
            add_dep_helper(b1.ins, ra_wr.ins, reason="ra dram write->read")
            add_dep_helper(b2.ins, ra_wr.ins, reason="ra dram write->read")
            nc.vector.tensor_mul(ouT_k[t][qt], ouT_k[t][qt], rb)

        def proj_unit(m, qt):
            """Proj accumulation k=0..5 for output tile (m, qt) + bias +
            output DMA. Only the k=5 matmul waits on pair 5's drain."""
            pool, tag = (mm_ps, "mm") if m % 2 == 0 else (st_ps, "st")
            ps = pool.tile([128, 512], f32, tag=tag, name=f"y_{m}_{qt}")
            for k in range(KT):
                nc.tensor.matmul(
                    ps,
                    wp_sb[:, k, m * 128:(m + 1) * 128],
                    ouT_k[k][qt],
                    start=(k == 0),
                    stop=(k == KT - 1),
                )
            yt = y_pool.tile([128, 512], f32, tag="y")
            nc.vector.tensor_scalar_add(yt, ps, bT_sb[:, m:m + 1])
            nc.scalar.dma_start(
                out=yT[m * 128:(m + 1) * 128, qt * 512:(qt + 1) * 512], in_=yt
            )

        hoist_store = {}

        def st_exp_j(t, j, pu_tiles):
            """ST matmuls + exp for (pair t, seq-tile j)."""
            he, ho = 2 * t, 2 * t + 1
            mt_q, mt_k = t, (H // 2) + t
            sts = {
                h: st_ps.tile([128, N], f32, tag="st", name=f"st_h{h}_j{j}")
                for h in (he, ho)
            }
            # alternate row groups (he: partitions 0-63, ho: 64-127)
            for qt in range(QT):
                for h, po in ((he, 0), (ho, 64)):
                    nc.tensor.matmul(
                        sts[h][:, qt * 512:(qt + 1) * 512],
                        qkT_sb[po:po + 64, mt_k, j * 128:(j + 1) * 128],
                        qkT_sb[po:po + 64, mt_q, qt * 512:(qt + 1) * 512],
                        start=True,
                        stop=True,
                    )
            for h in (he, ho):
                pu = put_pool.tile([128, N], bf16, tag="pu", name=f"pu_h{h}_j{j}")
                nc.scalar.activation(pu, sts[h], Exp, scale=SCALE)
                pu_tiles[(h, j)] = pu

        def attention_pair(t, extras_by_j=None):
            he, ho = 2 * t, 2 * t + 1
            # j=0's ST+exp may have been hoisted into the previous pair
            pu_tiles = hoist_store.pop(t, {})
            # sweep A: ST + exp (ACT-bound) + O for q-tile 0
            o_tiles = {
                h: o_ps.tile([65, 512], f32, tag="o", name=f"o_h{h}_q0")
                for h in (he, ho)
            }
            for j in range(NKT):
                if (he, j) not in pu_tiles:
                    st_exp_j(t, j, pu_tiles)
                if extras_by_j and j in extras_by_j:
                    for thunk in extras_by_j[j]:
                        thunk()
                for h in (he, ho):
                    nc.tensor.matmul(
                        o_tiles[h],
                        v_sb[:, j, h, :],
                        pu_tiles[(h, j)][:, 0:512],
                        start=(j == 0),
                        stop=(j == NKT - 1),
                    )
            drain_o(t, 0, o_tiles)
            # sweep B: O for q-tile 1 (re-reads retained PuT tiles)
            o_tiles2 = {
                h: o_ps.tile([65, 512], f32, tag="o", name=f"o_h{h}_q1")
                for h in (he, ho)
            }
            for j in range(NKT):
                for h in (he, ho):
                    nc.tensor.matmul(
                        o_tiles2[h],
                        v_sb[:, j, h, :],
                        pu_tiles[(h, j)][:, 512:1024],
                        start=(j == 0),
                        stop=(j == NKT - 1),
                    )
                if j == 3 and t < KT - 1:
                    # hoist the next pair's (j=0) ST+exp into this sweep so
                    # the boundary has no exp-latency bubble
                    nxt = hoist_store.setdefault(t + 1, {})
                    st_exp_j(t + 1, 0, nxt)
            drain_o(t, 1, o_tiles2)

        # ---- schedule: qk(t+1) and v chunks are emitted inside pair t's
        # j-loop so the PE fills ACT-bound gaps and qkT(t+1) is ready at the
        # pair boundary; pair 5's gaps hold the proj k=0..4 partials ----
        def extras(t):
            e = {j: [] for j in range(NKT)}
            if t == 0:
                for j in range(NKT):
                    e[j].append(lambda j=j: v_chunk(j, 0, 512))
            if t == 2:
                for j in range(NKT):
                    e[j].append(lambda j=j: v_chunk(j, 512, 256))
            if t < KT - 1:
                for u, j in enumerate((1, 3, 5, 7)):
                    e[j].append(lambda t=t, u=u: qk_unit(t + 1, u))
            return e

        for u in (0, 2, 1, 3):
            qk_unit(0, u)
        for t in range(KT):
            attention_pair(t, extras(t))
        # q-tile 0 first: its k=5 dep (drain(5,0)) resolves during sweep B,
        # and its matmuls cover the drain(5,1) chain before qt1's k=5
        for qt in range(QT):
            for m in range(KT):
                proj_unit(m, qt)

    nc.compile()
    return nc


def _get_nc():
    global _CACHED
    if _CACHED is None:
        _CACHED = _build()
    return _CACHED


def kernel(x, w_qkv, w_proj, b_proj):
    import ml_dtypes
    from concourse.bass_utils import run_bass_kernel_spmd

    x = np.asarray(x, dtype=np.float32)
    w_qkv = np.asarray(w_qkv, dtype=np.float32)
    w_proj = np.asarray(w_proj, dtype=np.float32)
    b_proj = np.asarray(b_proj, dtype=np.float32)

    nc = _get_nc()

    wqk_t = np.ascontiguousarray(
        w_qkv[:, : 2 * C].astype(ml_dtypes.bfloat16)
        .reshape(KT, 128, H, 128).transpose(2, 1, 0, 3)
    )
    wv = np.ascontiguousarray(w_qkv[:, 2 * C:].astype(ml_dtypes.bfloat16))
    wp = np.ascontiguousarray(w_proj.astype(ml_dtypes.bfloat16))
    bT = np.ascontiguousarray(b_proj.reshape(KT, 128).T)

    in_maps = []
    for b in range(B):
        in_maps.append(
            {
                "xT": np.ascontiguousarray(x[b].T.astype(ml_dtypes.bfloat16)),
                "wqk": wqk_t,
                "wv": wv,
                "wp": wp,
                "bT": bT,
            }
        )

    res = run_bass_kernel_spmd(nc, in_maps, list(range(NCORES)))
    out = np.empty((B, N, C), dtype=np.float32)
    for b in range(B):
        out[b] = res.results[b]["yT"].T
    return out



# revision 2
# speedup vs baseline: 1.0387x; 1.0387x over previous
"""Trainium2 Bass kernel for multi-head attention (B=8, N=1024, C=768, H=12, D=64).

Sharding: pure data parallelism - one batch element per NeuronCore (8 cores).

v2 dataflow ("natural-O"):
  - qk-pass: qkT = w_qk.T @ x.T -> [1536, 1024] (as 12 column tiles), bf16
  - v-pass:  v natural [1024, 768] + ones column per head -> v_sb [k, h, 65]
  - ST pass: ST[k, q] per head (contraction d=64), exp on ACT -> PuT bf16
  - O pass (NEW): natural orientation, per q-chunk of 128:
        O_nat[qc, 0:64 | l] = PuT_chunk.T @ [v_h | ones]   (65-wide matmuls)
    The softmax denominator l lands as PSUM COLUMN 64 -> per-partition
    scalars. Normalize = DVE reciprocal [128,4,1] + tensor_scalar_mul.
    No DRAM bounce, no cross-partition broadcast chains at all.
  - transpose: per (pair, chunk) one PE transpose matmul [128,128] via an
    identity rhs turns normalized O_nat (he|ho side by side) back into the
    O^T row layout proj needs; DVE copies PSUM->ouT bf16.
  - proj: yT = wp.T @ O^T; bias via ACT Identity activation (bias AP);
    output DMA on the scalar ring.

Schedule: per pair, sweep A = inline ST+exp (j=2..7; j=0,1 hoisted into the
previous pair's sweep B) + O chunks 0-3 + qk(t+1) units + transposes of the
previous pair's chunks 4-7; sweep B = O chunks 4-7 re-reading PuT + v(t+1)
units + 2 hoisted ST+exp of pair t+1 + transposes of chunks 0-3. Pair 5's
sweep B carries the first two proj units' k=0..4 partials so the final
drains hide under proj. Input DMAs are spread across the sync (xT), vector
(wqk) and gpsimd (wv/ident/bT/wp) rings so first-use tiles land ~2x sooner.
"""

import sys

sys.path.insert(0, "/opt/trn_rl_repo")

import numpy as np

B, N, C = 8, 1024, 768
H, D = 12, 64
SCALE = D ** -0.5  # 0.125
NCORES = 8
KT = C // 128      # 6 k-tiles over the C contraction
NKT = N // 128     # 8 k-tiles over sequence for attention
NP = H // 2        # 6 head pairs

_CACHED = None


def _build():
    from contextlib import ExitStack

    from concourse import bacc
    import concourse.bass as bass
    import concourse.mybir as mybir
    from concourse.tile import TileContext

    f32 = mybir.dt.float32
    bf16 = mybir.dt.bfloat16
    Exp = mybir.ActivationFunctionType.Exp
    Ident = mybir.ActivationFunctionType.Identity

    nc = bacc.Bacc("TRN2", target_bir_lowering=False, debug=False)

    xT = nc.dram_tensor("xT", [C, N], bf16, kind="ExternalInput").ap()
    wqk = nc.dram_tensor("wqk", [H, 128, KT, 128], bf16, kind="ExternalInput").ap()
    wv = nc.dram_tensor("wv", [C, C], bf16, kind="ExternalInput").ap()
    wp = nc.dram_tensor("wp", [C, C], bf16, kind="ExternalInput").ap()
    bT = nc.dram_tensor("bT", [128, KT], f32, kind="ExternalInput").ap()
    yT = nc.dram_tensor("yT", [C, N], f32, kind="ExternalOutput").ap()

    with TileContext(nc) as tc, ExitStack() as ctx:
        singles = ctx.enter_context(tc.tile_pool(name="singles", bufs=1))
        put_pool = ctx.enter_context(tc.tile_pool(name="put", bufs=24))
        nat_pool = ctx.enter_context(tc.tile_pool(name="nat", bufs=14))
        r_pool = ctx.enter_context(tc.tile_pool(name="r", bufs=4))
        y_pool = ctx.enter_context(tc.tile_pool(name="y", bufs=3))
        # PSUM: st (2 x 2 banks) + o (2 x 1 bank) + mm (2 x 1 bank) = 8 banks
        mm_ps = ctx.enter_context(tc.tile_pool(name="mm_ps", bufs=2, space="PSUM"))
        o_ps = ctx.enter_context(tc.tile_pool(name="o_ps", bufs=2, space="PSUM"))
        st_ps = ctx.enter_context(tc.tile_pool(name="st_ps", bufs=2, space="PSUM"))

        # ---- persistent SBUF ----
        xT_k = [
            [singles.tile([128, 512], bf16, name=f"xT_k{k}_q{qt}") for qt in range(2)]
            for k in range(KT)
        ]

        def xt(k, half):
            return xT_k[k][half]
        wqk_sb = [
            singles.tile([128, KT, 128], bf16, name=f"wqk_sb_{m}") for m in range(H)
        ]
        wv_lo = [singles.tile([128, 256], bf16, name=f"wv_lo{k}") for k in range(KT)]
        wv_hi = [singles.tile([128, 512], bf16, name=f"wv_hi{k}") for k in range(KT)]

        def wv_cols(k, t):
            return (
                wv_lo[k][:, t * 128:(t + 1) * 128]
                if t < 2
                else wv_hi[k][:, (t - 2) * 128:(t - 1) * 128]
            )
        wp_sb = singles.tile([128, KT, C], bf16)
        bT_sb = singles.tile([128, KT], f32)
        qkT_sb = singles.tile([128, H, N], bf16)       # 24 KB/part
        v_sb = singles.tile([128, NKT, H, 65], bf16)   # 12.2 KB/part
        ouT_k = [
            [singles.tile([128, 512], bf16, name=f"ouT_k{t}_q{qt}") for qt in range(2)]
            for t in range(KT)
        ]

        # ---- input DMAs on three parallel rings, first-use order ----
        # sync ring: xT qt0 (paces the k-major lead-in)
        for k in range(KT):
            nc.sync.dma_start(out=xT_k[k][0], in_=xT[k * 128:(k + 1) * 128, 0:512])
        # scalar ring: hot wqk tiles, xT qt1, cooler wqk (ACT idle early)
        for m in (0, 6):
            nc.scalar.dma_start(out=wqk_sb[m], in_=wqk[m])
        for k in range(KT):
            nc.scalar.dma_start(out=xT_k[k][1], in_=xT[k * 128:(k + 1) * 128, 512:1024])
        for m in (1, 7):
            nc.scalar.dma_start(out=wqk_sb[m], in_=wqk[m])
        # gpsimd ring, ordered by first use to keep early fabric bandwidth
        # for the critical sync/scalar transfers: wv for pairs 0-1, then the
        # rest of wv, then cooler wqk, bT, wp
        for k in range(KT):
            nc.gpsimd.dma_start(out=wv_lo[k], in_=wv[k * 128:(k + 1) * 128, 0:256])
        for m in (2, 8):
            nc.gpsimd.dma_start(out=wqk_sb[m], in_=wqk[m])
        for k in range(KT):
            nc.gpsimd.dma_start(out=wv_hi[k], in_=wv[k * 128:(k + 1) * 128, 256:768])
        for m in (3, 9, 4, 10, 5, 11):
            nc.gpsimd.dma_start(out=wqk_sb[m], in_=wqk[m])
        nc.gpsimd.dma_start(out=bT_sb, in_=bT[:, :])
        for k in range(KT):
            nc.gpsimd.dma_start(out=wp_sb[:, k, :], in_=wp[k * 128:(k + 1) * 128, :])

        # ones column for the softmax-denominator trick
        nc.vector.memset(v_sb[:, :, :, 64:65], 1.0)

        def qk_unit(t, u):
            """One (m, qt) unit of the qk-pass for head pair t (u in 0..3)."""
            m = t if u < 2 else (H // 2) + t
            qt = u % 2
            ps = mm_ps.tile([128, 512], f32, tag="mm", name=f"qk_{m}_{qt}")
            for k in range(KT):
                nc.tensor.matmul(
                    ps,
                    wqk_sb[m][:, k, :],
                    xt(k, qt),
                    start=(k == 0),
                    stop=(k == KT - 1),
                )
            nc.vector.tensor_copy(qkT_sb[:, m, qt * 512:(qt + 1) * 512], ps)

        def v_unit(t, j):
            """v[seq-tile j, head pair t] = x_j @ w_v[:, pair cols] (natural)."""
            ps = mm_ps.tile([128, 128], f32, tag="mm", name=f"v_{t}_{j}")
            for k in range(KT):
                nc.tensor.matmul(
                    ps,
                    xt(k, j // 4)[:, (j % 4) * 128:(j % 4 + 1) * 128],
                    wv_cols(k, t),
                    start=(k == 0),
                    stop=(k == KT - 1),
                )
            nc.vector.tensor_copy(
                v_sb[:, j, 2 * t:2 * t + 2, 0:64],
                ps.rearrange("p (h c) -> p h c", c=64),
            )

        def st_head(t, j, h, pu_tiles):
            """ST matmuls + exp for one head of (pair t, seq-tile j)."""
            mt_q, mt_k = t, (H // 2) + t
            po = 0 if h % 2 == 0 else 64
            st = st_ps.tile([128, N], f32, tag="st", name=f"st_h{h}_j{j}")
            for qt in range(2):
                nc.tensor.matmul(
                    st[:, qt * 512:(qt + 1) * 512],
                    qkT_sb[po:po + 64, mt_k, j * 128:(j + 1) * 128],
                    qkT_sb[po:po + 64, mt_q, qt * 512:(qt + 1) * 512],
                    start=True,
                    stop=True,
                )
            pu = put_pool.tile([128, N], bf16, tag="pu", name=f"pu_h{h}_j{j}")
            nc.scalar.activation(pu, st, Exp, scale=SCALE)
            pu_tiles[(h, j)] = pu

        def o_nat_half(t, j, half, ot, h, pu_tiles):
            """Natural-O matmuls for one head, seq-tile j, 4 q-chunks.

            PSUM accumulation groups are bank-granular (2KB zero regions), so
            each head's [128, 4, 128] tile is one bank with a single
            start/stop: start on the first matmul (j=0, c4=0), stop on the
            last (j=7, c4=3). Intermediate chunks' first writes land on
            pending-zero bytes, which the PE overwrites (zero+add)."""
            for c4 in range(4):
                c = half * 4 + c4
                nc.tensor.matmul(
                    ot[:, c4, 0:65],
                    pu_tiles[(h, j)][:, c * 128:(c + 1) * 128],
                    v_sb[:, j, h, :],
                    start=(j == 0 and c4 == 0),
                    stop=(j == NKT - 1 and c4 == 3),
                )

        def drain_nat(t, half, o_he, o_ho, nat_tiles):
            """Per-partition normalize: r = 1/l (col 64), nat = O * r (bf16),
            then XBAR DMA transposes on the (idle) gpsimd ring move each
            normalized chunk into the ouT row layout proj needs - no PE or
            DVE involvement."""
            he, ho = 2 * t, 2 * t + 1
            for h, ot, col0 in ((he, o_he, 0), (ho, o_ho, 64)):
                r = r_pool.tile([128, 4, 1], f32, tag="r", name=f"r_{h}_{half}")
                nc.vector.reciprocal(r, ot[:, :, 64:65])
                for c4 in range(4):
                    c = half * 4 + c4
                    nc.vector.tensor_scalar_mul(
                        nat_tiles[c][:, col0:col0 + 64], ot[:, c4, 0:64], r[:, c4, :]
                    )
            for c4 in range(4):
                c = half * 4 + c4
                nc.sync.dma_start_transpose(
                    out=ouT_k[t][half][:, c4 * 128:(c4 + 1) * 128],
                    in_=nat_tiles[c],
                )

        def proj_mms(ps, m, qt, ks):
            for i, k in enumerate(ks):
                nc.tensor.matmul(
                    ps,
                    wp_sb[:, k, m * 128:(m + 1) * 128],
                    ouT_k[k][qt],
                    start=(k == 0),
                    stop=(k == KT - 1),
                )

        def proj_tail(ps, m, qt):
            yt = y_pool.tile([128, 512], f32, tag="y")
            nc.scalar.activation(yt, ps, Ident, bias=bT_sb[:, m:m + 1])
            nc.scalar.dma_start(
                out=yT[m * 128:(m + 1) * 128, qt * 512:(qt + 1) * 512], in_=yt
            )

        def proj_alloc(m, qt):
            pool, tag = (mm_ps, "mm") if m % 2 == 0 else (st_ps, "st")
            return pool.tile([128, 512], f32, tag=tag, name=f"y_{m}_{qt}")

        def proj_unit(m, qt):
            ps = proj_alloc(m, qt)
            proj_mms(ps, m, qt, range(KT))
            proj_tail(ps, m, qt)

        hoist_store = {}

        def make_qk_spiller(t1, units):
            """Returns spill(n): emits the next n matmuls of pair-t1 qk units
            (fine-grained so the in-order PE queue never blocks STs for long).
            Handles PSUM alloc at unit start and the qkT cast at unit end."""
            steps = [(u, k) for u in units for k in range(KT)]
            state = {"i": 0, "ps": None}

            def spill(n):
                for _ in range(n):
                    if state["i"] >= len(steps):
                        return
                    u, k = steps[state["i"]]
                    m = t1 if u < 2 else (H // 2) + t1
                    qt = u % 2
                    if k == 0:
                        state["ps"] = mm_ps.tile(
                            [128, 512], f32, tag="mm", name=f"qk_{m}_{qt}"
                        )
                    nc.tensor.matmul(
                        state["ps"],
                        wqk_sb[m][:, k, :],
                        xt(k, qt),
                        start=(k == 0),
                        stop=(k == KT - 1),
                    )
                    if k == KT - 1:
                        nc.vector.tensor_copy(
                            qkT_sb[:, m, qt * 512:(qt + 1) * 512], state["ps"]
                        )
                        state["ps"] = None
                    state["i"] += 1

            return spill

        def sweep_A(t, pu_tiles, o_he, o_ho, qk_spill):
            """j-blocks: block j = [O_A(he,j) | v(j) | ST(he,j+2) | qk spill |
            O_A(ho,j) | ST(ho,j+2)] - STs just-in-time so the exp stream on
            ACT never starves, with fine-grained filler between them."""
            he, ho = 2 * t, 2 * t + 1
            if t == 0:
                for j01 in (0, 1):
                    st_head(0, j01, he, pu_tiles)
                    st_head(0, j01, ho, pu_tiles)
            nxt = hoist_store.setdefault(t + 1, {}) if t < NP - 1 else None
            for j in range(NKT):
                v_unit(t, j)
                o_nat_half(t, j, 0, o_he, he, pu_tiles)
                if j < NKT - 2:
                    st_head(t, j + 2, he, pu_tiles)
                elif j == NKT - 1 and t < NP - 1:
                    # hoist: next pair's (j=0, he) ST right after exp(t,7,he)
                    # frees its st buf, so the exp stream never pauses at the
                    # pair boundary
                    st_head(t + 1, 0, 2 * t + 2, nxt)
                qk_spill(3)
                o_nat_half(t, j, 0, o_ho, ho, pu_tiles)
                if j < NKT - 2:
                    st_head(t, j + 2, ho, pu_tiles)
                elif j == NKT - 1 and t < NP - 1:
                    st_head(t + 1, 0, 2 * t + 3, nxt)

        def sweep_B(t, pu_tiles, nat_tiles, proj_partials, qk_spill):
            o_he = o_ps.tile([128, 4, 128], f32, tag="o", name=f"oB_{2 * t}")
            o_ho = o_ps.tile([128, 4, 128], f32, tag="o", name=f"oB_{2 * t + 1}")
            he, ho = 2 * t, 2 * t + 1
            nxt = hoist_store.get(t + 1) if t < NP - 1 else None
            if t < NP - 1:
                qk_spill(6)
                st_head(t + 1, 1, 2 * t + 2, nxt)
                st_head(t + 1, 1, 2 * t + 3, nxt)
            else:
                ps0 = st_ps.tile([128, 512], f32, tag="st", name="y_0_0")
                proj_mms(ps0, 0, 0, range(KT - 1))
                ps1 = st_ps.tile([128, 512], f32, tag="st", name="y_1_0")
                proj_mms(ps1, 1, 0, range(KT - 1))
                proj_partials.extend([(ps0, 0), (ps1, 1)])
            for j in range(NKT):
                o_nat_half(t, j, 1, o_he, he, pu_tiles)
                o_nat_half(t, j, 1, o_ho, ho, pu_tiles)
            return o_he, o_ho

        def qk_lead_in():
            """Initial qk units, k-major so the serialized xT DMAs pace the
            PE instead of stalling it."""
            for us in ((0, 2), (1, 3)):
                ps = {}
                for u in us:
                    m = 0 if u < 2 else (H // 2)
                    ps[u] = mm_ps.tile([128, 512], f32, tag="mm", name=f"qk0_{m}_{u % 2}")
                for k in range(KT):
                    for u in us:
                        m = 0 if u < 2 else (H // 2)
                        nc.tensor.matmul(
                            ps[u],
                            wqk_sb[m][:, k, :],
                            xt(k, u % 2),
                            start=(k == 0),
                            stop=(k == KT - 1),
                        )
                for u in us:
                    m = 0 if u < 2 else (H // 2)
                    qt = u % 2
                    nc.vector.tensor_copy(
                        qkT_sb[:, m, qt * 512:(qt + 1) * 512], ps[u]
                    )

        # ---- schedule ----
        qk_lead_in()
        proj_partials = []
        for t in range(NP):
            pu_tiles = hoist_store.pop(t, {})
            qk_spill = (
                make_qk_spiller(t + 1, (0, 2, 1, 3))
                if t < NP - 1
                else (lambda n: None)
            )
            o_heA = o_ps.tile([128, 4, 128], f32, tag="o", name=f"oA_{2 * t}")
            o_hoA = o_ps.tile([128, 4, 128], f32, tag="o", name=f"oA_{2 * t + 1}")
            nat_tiles = [
                nat_pool.tile([128, 128], bf16, tag="nat", name=f"nat_{t}_{c}")
                for c in range(8)
            ]
            sweep_A(t, pu_tiles, o_heA, o_hoA, qk_spill)
            drain_nat(t, 0, o_heA, o_hoA, nat_tiles)
            o_heB, o_hoB = sweep_B(t, pu_tiles, nat_tiles, proj_partials, qk_spill)
            drain_nat(t, 1, o_heB, o_hoB, nat_tiles)

        # ---- proj tail: qt0 first; ouT tiles arrive via gpsimd XBAR
        # transposes emitted inside the drains ----
        for ps, m in proj_partials:
            proj_mms(ps, m, 0, [KT - 1])
            proj_tail(ps, m, 0)
        for m in range(2, KT):
            proj_unit(m, 0)
        for m in range(KT):
            proj_unit(m, 1)

    nc.compile()
    return nc


def _get_nc():
    global _CACHED
    if _CACHED is None:
        _CACHED = _build()
    return _CACHED


def _host_inputs(x, w_qkv, w_proj, b_proj):
    import ml_dtypes

    wqk_t = np.ascontiguousarray(
        w_qkv[:, : 2 * C].astype(ml_dtypes.bfloat16)
        .reshape(KT, 128, H, 128).transpose(2, 1, 0, 3)
    )
    wv = np.ascontiguousarray(w_qkv[:, 2 * C:].astype(ml_dtypes.bfloat16))
    wp = np.ascontiguousarray(w_proj.astype(ml_dtypes.bfloat16))
    bT = np.ascontiguousarray(b_proj.reshape(KT, 128).T)

    in_maps = []
    for b in range(B):
        in_maps.append(
            {
                "xT": np.ascontiguousarray(x[b].T.astype(ml_dtypes.bfloat16)),
                "wqk": wqk_t,
                "wv": wv,
                "wp": wp,
                "bT": bT,
            }
        )
    return in_maps


def kernel(x, w_qkv, w_proj, b_proj):
    from concourse.bass_utils import run_bass_kernel_spmd

    x = np.asarray(x, dtype=np.float32)
    w_qkv = np.asarray(w_qkv, dtype=np.float32)
    w_proj = np.asarray(w_proj, dtype=np.float32)
    b_proj = np.asarray(b_proj, dtype=np.float32)

    nc = _get_nc()
    in_maps = _host_inputs(x, w_qkv, w_proj, b_proj)
    res = run_bass_kernel_spmd(nc, in_maps, list(range(NCORES)))
    out = np.empty((B, N, C), dtype=np.float32)
    for b in range(B):
        out[b] = res.results[b]["yT"].T
    return out
